# revision 9
# baseline (speedup 1.0000x reference)
"""Trainium2 Bass kernel for nn_ColorDecoder (segment_reduce).

Reference computation (per sample):
  logits = conv1x1(feature_map)            [21, 64, 64]
  seg    = softmax_k(logits)
  seg_up = bilinear_upsample(seg, 512, 512)          (never materialized!)
  q      = einsum('chw,khw->kc', x, seg_up) / (H*W)  [21, 3]
  attn   = einsum('chw,kc->khw', x, q)               [21, 512, 512]

Key algebraic trick: bilinear upsampling U is linear, so
  q[k,c] = sum_hw seg[k,hw] * (U_y^T x_c U_x)[hw] / (H*W)
which needs only the 64x64 adjoint-downsampled x — the 512x512 seg_up is
never computed.  The output attn is a rank-3 broadcast computed by a
block-diagonal PE matmul (6 spatial chunks x 21 classes packed into 126
PSUM partitions).

Sharding: pure data parallel, batch 16 -> 2 samples on each of 8 cores.
"""

import numpy as np

import bass_rust
import concourse.bass as bass
import concourse.mybir as mybir
from concourse.ap import AP
from concourse.tile import TileContext, ScopedClock
from concourse.bass_utils import run_bass_kernel_spmd

# ---------------------------------------------------------------------------
# Workaround for this walrus build: instructions carrying more than one
# semaphore wait fail codegen ("Too many sync wait commands").  Hoist excess
# waits onto preceding same-engine InstNoOps; same for the end-of-kernel
# drain.
# ---------------------------------------------------------------------------
_MAX_WAITS = 1
_orig_commit = TileContext._commit_instruction


def _commit_split(self, inst, lazy_reg_writes: bool = True):
    si = getattr(inst, "sync_info", None)
    if si is not None and len(si.on_wait) > _MAX_WAITS:
        waits = list(si.on_wait)
        extra, keep = waits[:-_MAX_WAITS], waits[-_MAX_WAITS:]
        for wt in extra:
            nop = mybir.InstNoOp(
                name=self.nc.get_next_instruction_name(),
                sync_info=mybir.SyncInfo(on_wait=[wt], on_update=[]),
                bass_nofuse=True,
                engine=inst.engine,
            )
            _orig_commit(self, nop, lazy_reg_writes)
        inst.sync_info = mybir.SyncInfo(on_wait=keep, on_update=list(si.on_update))
    return _orig_commit(self, inst, lazy_reg_writes)


def _patched_drain_and_barrier(self, tick_clock, wait_clock):
    drain_inst = self.nc.sync.drain()
    wait_clock.add_sem_waits(
        drain_inst.ins, ScopedClock({None: tick_clock.global_clock})
    )
    si = drain_inst.ins.sync_info
    waits = list(si.on_wait) if si else []
    if len(waits) > _MAX_WAITS:
        drain_inst.ins.sync_info = bass_rust.SyncInfo(on_wait=[], on_update=[])
        by_name = {hh.name: hh for hh in self.sems.allocated().values()}
        for wt in waits:
            self.nc.sync.nop().wait_op(by_name[wt.ant_name], wt.wait_value, "sem-ge")
    self.nc.all_engine_barrier()
    assert self.sems is not None
    popped = self.nc._tile_sem_poison_stack.pop()
    assert popped is self._sem_poison
    self.nc.clear_and_free_semaphores(list(self.sems.allocated().values()))
    self.nc.all_engine_barrier()


TileContext._commit_instruction = _commit_split
TileContext._drain_and_barrier = _patched_drain_and_barrier

# ---------------------------------------------------------------------------
# Problem geometry (hardcoded per spec)
# ---------------------------------------------------------------------------
B, F, SH, SW = 16, 256, 64, 64      # feature map
H, W = 512, 512                     # image
K = 21                              # classes
NCORES = 8
BPC = B // NCORES                   # samples per core = 2
HW = H * W                          # 262144
SHW = SH * SW                       # 4096
NREP = 6                            # spatial chunks in the attn matmul
ROWS = (86, 86, 86, 86, 86, 82)     # image rows per chunk (sum = 512)
ROW0 = (0, 86, 172, 258, 344, 430)
NGRP = 86                           # col-groups of 512 in the widest chunk
GB = 8                              # groups per XI window
GBS = 8                             # groups per output store batch
NW = (NGRP + GB - 1) // GB          # XI windows = 11

F32 = mybir.dt.float32
F32R = mybir.dt.float32r


def _upsample_matrix(n_in, n_out):
    """align_corners=True bilinear interpolation matrix [n_out, n_in]."""
    u = np.zeros((n_out, n_in), dtype=np.float64)
    pos = np.linspace(0.0, n_in - 1.0, n_out)
    i0 = np.floor(pos).astype(np.int64)
    i1 = np.minimum(i0 + 1, n_in - 1)
    frac = pos - i0
    np.add.at(u, (np.arange(n_out), i0), 1.0 - frac)
    np.add.at(u, (np.arange(n_out), i1), frac)
    return u.astype(np.float32)


def _host_consts(conv_w, conv_b):
    uy = _upsample_matrix(SH, H)            # [512, 64]
    ux = _upsample_matrix(SW, W)            # [512, 64]
    idn64 = np.eye(64, dtype=np.float32)
    # attn contraction row index is r = 6*c + i  (c channel, i spatial chunk)
    i3r = np.zeros((3, 3 * NREP), dtype=np.float32)
    for i in range(NREP):
        for c in range(3):
            i3r[c, NREP * c + i] = 1.0
    mask = np.zeros((3 * NREP, K * NREP), dtype=np.float32)
    for i in range(NREP):
        for c in range(3):
            mask[NREP * c + i, K * i : K * i + K] = 1.0
    return {
        "convwT": np.ascontiguousarray(conv_w.T),      # [256, 21]
        "convb": conv_b.reshape(1, K).astype(np.float32),
        "uy": uy,
        "ux": ux,
        "idn64": idn64,
        "i3r": i3r,
        "maskblk": mask,
        "ones64": np.ones((1, 64), dtype=np.float32),
    }


def _build(with_bias: bool, loop: int = 1, debug: bool = False):
    nc = bass.Bass("TRN2", target_bir_lowering=False, debug=False)

    fm_d = nc.dram_tensor("fm", [BPC, F, SHW], F32, kind="ExternalInput").ap()
    x_d = nc.dram_tensor("x", [BPC, 3, HW], F32, kind="ExternalInput").ap()
    convwT_d = nc.dram_tensor("convwT", [F, K], F32, kind="ExternalInput").ap()
    convb_d = nc.dram_tensor("convb", [1, K], F32, kind="ExternalInput").ap()
    uy_d = nc.dram_tensor("uy", [H, SH], F32, kind="ExternalInput").ap()
    ux_d = nc.dram_tensor("ux", [W, SW], F32, kind="ExternalInput").ap()
    idn_d = nc.dram_tensor("idn64", [64, 64], F32, kind="ExternalInput").ap()
    i3r_d = nc.dram_tensor("i3r", [3, 3 * NREP], F32, kind="ExternalInput").ap()
    mask_d = nc.dram_tensor("maskblk", [3 * NREP, K * NREP], F32,
                            kind="ExternalInput").ap()
    ones_d = nc.dram_tensor("ones64", [1, 64], F32, kind="ExternalInput").ap()
    out_d = nc.dram_tensor("attn", [BPC, K, HW], F32, kind="ExternalOutput").ap()
    if debug:
        dbg_qt = nc.dram_tensor("dbg_qt", [BPC, 3, K], F32, kind="ExternalOutput").ap()
        dbg_sall = nc.dram_tensor("dbg_sall", [BPC, 128, 32], F32, kind="ExternalOutput").ap()
        dbg_xsn = nc.dram_tensor("dbg_xsn", [BPC, 128, 96], F32, kind="ExternalOutput").ap()
        dbg_wp = nc.dram_tensor("dbg_wp", [BPC, 18, K * NREP], F32, kind="ExternalOutput").ap()
        dbg_e2 = nc.dram_tensor("dbg_e2", [BPC, 128, 32 * K], F32, kind="ExternalOutput").ap()
        dbg_xi0 = nc.dram_tensor("dbg_xi0", [3 * NREP, GB * W], F32, kind="ExternalOutput").ap()
        dbg_stg0 = nc.dram_tensor("dbg_stg0", [128, GB * W], F32, kind="ExternalOutput").ap()

    with TileContext(nc) as tc:
        with (
            tc.tile_pool(name="const", bufs=1) as cpool,
            tc.tile_pool(name="fm", bufs=1) as fmpool,
            tc.tile_pool(name="xc", bufs=2) as xcpool,
            tc.tile_pool(name="seg", bufs=2) as segpool,
            tc.tile_pool(name="xi", bufs=3) as xipool,
            tc.tile_pool(name="stg", bufs=2) as stgpool,
            tc.tile_pool(name="small", bufs=2) as smpool,
            tc.tile_pool(name="ps1", bufs=2, space="PSUM") as ps1,
            tc.tile_pool(name="psw", bufs=3, space="PSUM") as psw,
            tc.tile_pool(name="psa", bufs=3, space="PSUM") as psa,
        ):
            # ---- constants (loaded once) ----
            convwT_s = cpool.tile([128, F // 128, K], F32, tag="convwT")
            nc.sync.dma_start(
                out=convwT_s[:], in_=convwT_d.rearrange("(a p) k -> p a k", p=128)
            )
            convb_s = cpool.tile([1, K], F32, tag="convb")
            nc.sync.dma_start(out=convb_s[:], in_=convb_d[:])
            ones_s = cpool.tile([1, 64], F32, tag="ones64")
            nc.sync.dma_start(out=ones_s[:], in_=ones_d[:])
            uy_s = cpool.tile([128, 4, SH], F32R, tag="uy")
            nc.gpsimd.dma_start(
                out=uy_s[:], in_=uy_d.rearrange("(p a) k -> p a k", a=4)
            )
            ux_s = cpool.tile([128, 4, SW], F32, tag="ux")
            nc.sync.dma_start(
                out=ux_s[:], in_=ux_d.rearrange("(a p) k -> p a k", p=128)
            )
            idn_s = cpool.tile([64, 64], F32, tag="idn64")
            nc.sync.dma_start(out=idn_s[:], in_=idn_d[:])
            i3r_s = cpool.tile([3, 3 * NREP], F32, tag="i3r")
            nc.sync.dma_start(out=i3r_s[:], in_=i3r_d[:])
            mask_s = cpool.tile([3 * NREP, K * NREP], F32, tag="maskblk")
            nc.sync.dma_start(out=mask_s[:], in_=mask_d[:])

            def one_iteration():
                for b in range(BPC):
                    sample(b)

            def sample(b):
                # ---- load feature map: two [128, 4096] tiles ----
                fm_s = fmpool.tile([128, 2, SHW], F32, tag="fm")
                nc.gpsimd.dma_start(
                    out=fm_s[:], in_=fm_d[b].rearrange("(a p) n -> p a n", p=128)
                )

                # ---- load x in blocked H-partition layout (fp32r): partition p
                # holds rows 4p..4p+3 -> one 8KB contiguous run per (p, c) ----
                xc_s = xcpool.tile([128, 3, 4, W], F32R, tag="xc")
                nc.gpsimd.dma_start(
                    out=xc_s[:],
                    in_=x_d[b].rearrange("c (p q w) -> p c q w", q=4, w=W),
                )

                # ---- stage 1: logits -> exp, hw-on-partition pairs ----
                e2_s = segpool.tile([128, 32 * K], F32, tag="e2")
                s_all = smpool.tile([128, 32], F32, tag="sall")
                for pair in range(32):
                    lp = ps1.tile([128, K], F32, tag="logit")
                    for half in range(2):  # chunk 2*pair (+half)
                        col0 = 128 * pair + 64 * half
                        for kc in range(2):
                            nc.tensor.matmul(
                                lp[64 * half : 64 * half + 64, :],
                                fm_s[:, kc, col0 : col0 + 64],
                                convwT_s[:, kc, :],
                                start=(kc == 0),
                                stop=(kc == 1) and not with_bias,
                                tile_position=(0, 64 * half),
                            )
                        if with_bias:
                            nc.tensor.matmul(
                                lp[64 * half : 64 * half + 64, :],
                                ones_s[:],
                                convb_s[:],
                                start=False,
                                stop=True,
                                tile_position=(0, 64 * half),
                            )
                    nc.scalar.activation(
                        e2_s[:, K * pair : K * pair + K],
                        lp[:],
                        mybir.ActivationFunctionType.Exp,
                        accum_out=s_all[:, pair : pair + 1],
                    )

                # ---- downsample x: xs = U_y^T x U_x in (dh,w)-partition layout --
                xsn_s = smpool.tile([128, 96], F32, tag="xsn")
                for c in range(3):
                    tp = psw.tile([64, W], F32, tag="w")
                    for q in range(4):
                        nc.tensor.matmul(
                            tp[:],
                            uy_s[:, q, :],
                            xc_s[:, c, q, :],
                            start=(q == 0),
                            stop=(q == 3),
                        )
                    t_s = smpool.tile([64, W], F32, tag="tsb")
                    nc.vector.tensor_copy(t_s[:], tp[:])
                    tT_s = smpool.tile([128, 4 * 64], F32, tag="ttsb")
                    for q in range(4):
                        tTp = psw.tile([128, 64], F32, tag="w")
                        nc.tensor.transpose(
                            tTp[:], t_s[:, 128 * q : 128 * q + 128], idn_s[:]
                        )
                        nc.vector.tensor_copy(
                            tT_s[:, 64 * q : 64 * q + 64], tTp[:]
                        )
                    xsp = psw.tile([128, 32], F32, tag="w")
                    for dlt in range(2):
                        for q in range(4):
                            nc.tensor.matmul(
                                xsp[64 * dlt : 64 * dlt + 64, :],
                                ux_s[:, q, :],
                                tT_s[:, 64 * q + dlt : 64 * q + 64 : 2],
                                start=(q == 0),
                                stop=(q == 3),
                                tile_position=(0, 64 * dlt),
                                skip_group_check=True,
                            )
                    nc.vector.tensor_copy(xsn_s[:, 32 * c : 32 * c + 32], xsp[:])

                # ---- softmax denominators folded into xs ----
                r_all = smpool.tile([128, 32], F32, tag="rall")
                nc.vector.reciprocal(r_all[:], s_all[:])
                nc.vector.tensor_scalar_mul(r_all[:], r_all[:], 1.0 / HW)
                for c in range(3):
                    nc.vector.tensor_mul(
                        xsn_s[:, 32 * c : 32 * c + 32],
                        xsn_s[:, 32 * c : 32 * c + 32],
                        r_all[:],
                    )

                # ---- q^T [3, 21] ----
                qtp = psw.tile([3, K], F32, tag="w")
                for pair in range(32):
                    nc.tensor.matmul(
                        qtp[:],
                        xsn_s[:, pair : 96 : 32],
                        e2_s[:, K * pair : K * pair + K],
                        start=(pair == 0),
                        stop=(pair == 31),
                    )
                qt_s = smpool.tile([3, K], F32, tag="qtsb")
                nc.scalar.copy(qt_s[:], qtp[:])
                if debug:
                    nc.sync.dma_start(out=dbg_qt[b], in_=qt_s[:])
                    nc.sync.dma_start(out=dbg_sall[b], in_=s_all[:])
                    nc.sync.dma_start(out=dbg_xsn[b], in_=xsn_s[:])
                    nc.sync.dma_start(out=dbg_e2[b], in_=e2_s[:])

                # ---- W_pack [18, 126] = blockdiag(q^T x6), fp32r ----
                wrp = psw.tile([3 * NREP, K * NREP], F32, tag="w")
                nc.tensor.matmul(
                    wrp[:],
                    i3r_s[:],
                    qt_s[:].unsqueeze(1).broadcast_to((3, NREP, K)),
                    start=True,
                    stop=True,
                )
                wpack_s = smpool.tile([3 * NREP, K * NREP], F32R, tag="wpack")
                nc.vector.tensor_mul(wpack_s[:], wrp[:], mask_s[:])
                if debug:
                    nc.sync.dma_start(
                        out=dbg_wp[b], in_=wpack_s[:].bitcast(F32)
                    )

                # ---- attn: block-diag matmul over 86 col-groups ----
                # one XI window per store batch (GB == GBS); xi loads are
                # single 3D-AP HWDGE DMAs issued 2 windows ahead of use so
                # they sit in front of the fat stores in each engine FIFO.
                xi_tiles = {}
                load_xi(b, 0, xi_tiles)
                load_xi(b, 1, xi_tiles)
                for sb0 in range(0, NGRP, GBS):
                    wdw = sb0 // GBS
                    if wdw + 2 < NW:
                        load_xi(b, wdw + 2, xi_tiles)
                    sba = min(GBS, NGRP - sb0)          # strips 0-4
                    sbb = max(0, min(GBS, ROWS[5] - sb0))  # strip 5
                    stg_s = stgpool.tile([128, GBS * W], F32, tag="stg")
                    xi_s = xi_tiles.pop(wdw)
                    for g in range(sb0, sb0 + sba):
                        ap_ = psa.tile([128, W], F32, tag="attnps")
                        nc.tensor.matmul(
                            ap_[0 : K * NREP, :],
                            wpack_s[:],
                            xi_s[:, (g - sb0) * W : (g - sb0) * W + W],
                            start=True,
                            stop=True,
                        )
                        dst = stg_s[0 : K * NREP, (g - sb0) * W : (g - sb0) * W + W]
                        nc.vector.tensor_copy(dst, ap_[0 : K * NREP, :])
                    steng = nc.sync
                    if sbb == sba:
                        # uniform batch: all 6 strips x 21 classes in one store
                        dst = AP(
                            tensor=out_d.tensor,
                            offset=b * K * HW + sb0 * W,
                            ap=[[ROWS[0] * W, NREP], [HW, K], [1, sba * W]],
                        )
                        steng.dma_start(out=dst, in_=stg_s[0 : K * NREP, : sba * W])
                    else:
                        dst = AP(
                            tensor=out_d.tensor,
                            offset=b * K * HW + sb0 * W,
                            ap=[[ROWS[0] * W, 5], [HW, K], [1, sba * W]],
                        )
                        steng.dma_start(out=dst, in_=stg_s[0 : 5 * K, : sba * W])
                        if sbb > 0:
                            dstb = AP(
                                tensor=out_d.tensor,
                                offset=b * K * HW + ROW0[5] * W + sb0 * W,
                                ap=[[HW, K], [1, sbb * W]],
                            )
                            steng.dma_start(
                                out=dstb, in_=stg_s[5 * K : 6 * K, : sbb * W]
                            )

            def load_xi(b, wdw, xi_tiles):
                g0 = wdw * GB
                ga = min(GB, NGRP - g0)             # groups for strips 0-4
                gb = max(0, min(GB, ROWS[5] - g0))  # groups for strip 5
                # partition r = 6*c + i holds x[b, c, strip-i rows g0..g0+ga)
                xi_s = xipool.tile([3 * NREP, GB * W], F32R, tag="xi")
                ldeng = nc.gpsimd
                for c in range(3):
                    if gb == ga:
                        # uniform window: strips 0-5 in one 2D-AP DMA
                        ldeng.dma_start(
                            out=xi_s[NREP * c : NREP * c + 6, : ga * W],
                            in_=AP(
                                tensor=x_d.tensor,
                                offset=b * 3 * HW + c * HW + g0 * W,
                                ap=[[ROWS[0] * W, 6], [1, ga * W]],
                            ),
                        )
                    else:
                        ldeng.dma_start(
                            out=xi_s[NREP * c : NREP * c + 5, : ga * W],
                            in_=AP(
                                tensor=x_d.tensor,
                                offset=b * 3 * HW + c * HW + g0 * W,
                                ap=[[ROWS[0] * W, 5], [1, ga * W]],
                            ),
                        )
                        if gb > 0:
                            ldeng.dma_start(
                                out=xi_s[NREP * c + 5 : NREP * c + 6, : gb * W],
                                in_=AP(
                                    tensor=x_d.tensor,
                                    offset=b * 3 * HW + c * HW + ROW0[5] * W + g0 * W,
                                    ap=[[1, gb * W]],
                                ),
                            )
                        if gb < ga:
                            # fill the strip-5 tail (never stored) with finite
                            # in-bounds data so 0*garbage can't make NaNs
                            ldeng.dma_start(
                                out=xi_s[NREP * c + 5 : NREP * c + 6, gb * W : ga * W],
                                in_=AP(
                                    tensor=x_d.tensor,
                                    offset=b * 3 * HW + c * HW,
                                    ap=[[1, (ga - gb) * W]],
                                ),
                            )
                xi_tiles[wdw] = xi_s

            for _ in range(loop):
                one_iteration()

    return nc


_cache: dict = {}


def _get_nc(with_bias: bool, loop: int, debug: bool = False):
    key = (with_bias, loop, debug)
    if key not in _cache:
        _cache[key] = _build(with_bias, loop, debug)
    return _cache[key]


def kernel(feature_map, x, conv_w, conv_b, _loop: int = 1, _debug: bool = False):
    feature_map = np.ascontiguousarray(feature_map, dtype=np.float32)
    x = np.ascontiguousarray(x, dtype=np.float32)
    conv_w = np.ascontiguousarray(conv_w, dtype=np.float32)
    conv_b = np.ascontiguousarray(conv_b, dtype=np.float32)

    with_bias = bool(np.any(conv_b != 0.0))
    nc = _get_nc(with_bias, _loop, _debug)
    consts = _host_consts(conv_w, conv_b)

    in_maps = []
    for core in range(NCORES):
        b0 = core * BPC
        in_maps.append(
            {
                "fm": feature_map[b0 : b0 + BPC].reshape(BPC, F, SHW),
                "x": x[b0 : b0 + BPC].reshape(BPC, 3, HW),
                **consts,
            }
        )
    res = run_bass_kernel_spmd(nc, in_maps, list(range(NCORES)))
    out = np.concatenate(
        [res.results[i]["attn"].reshape(BPC, K, H, W) for i in range(NCORES)],
        axis=0,
    )
    if _debug:
        return out, res.results
    return out

